# revision 76
# baseline (speedup 1.0000x reference)
"""GAT kernel for Trainium2, SPMD over 8 NeuronCores.

Math: the reference GAT variant computes attention logits e[b,h,i,j] that do
NOT depend on j (the "untransposed Wh2" formulation), so softmax over a row
whose support (adj!=0) carries a constant value collapses to 1/deg(i) on the
support and 0 elsewhere (NEG_INF -> exp underflow -> exactly 0 in fp32).
Hence, per batch element b:

    out[b] = elu( diag(1/deg_b) @ (adj_b * adj_weight_b) @ (h_b @ W) )

with deg_b[i] = sum_j adj_b[i,j].  The result is head-independent and `a` is
unused.  Sharding: data-parallel over batch (B == n_cores == 8).

Schedule (v5):
 - W_d and h_d^T are concatenated host-side into one [1024, 1536] tensor so
   each contraction block d arrives as ONE dma (8 descriptor gens, d-order
   arrival); adj/adj_weight stream strictly after.
 - Tiny warmup matmuls on a gpsimd-memset tile trip the HAM clock gate
   before real data lands.
 - MM1 64 + MM2 64/2 matmuls at 512 cols; evac via DVE+ACT in j-order.
 - deg = DVE partial adds over adjT + 4 one-column PE matmuls vs ones.
 - ELU tail: one ACT exp + one custom DVE op per tile:
       out = min(exp_t - 1, relu(psum * r))
   (|r*x| < 0.5 on this data so exp never overflows f16).
 - Output f16 (upcast on host), store DMAs on the gpsimd queue.
"""

import os

import numpy as np

import concourse.bass as bass
import concourse.tile as tile
from concourse import bacc, mybir
from concourse.bass import ts
from concourse.bass_utils import run_bass_kernel_spmd

# ---- custom DVE op: ELU tail ---------------------------------------------
import concourse.dve_ops as dve_ops
from concourse.dve_ops import DveOp, OPS
from concourse.dve_spec import Spec, Src0, Src1, C0, One, relu, minn, lower
from concourse.dve_uop import DveOpSpec


def _register_elu_tail():
    name = "ELU_TAIL_ANT"
    for op in OPS:
        if op.name == name:
            return op
    spec = Spec(
        body=minn(Src0 - One, relu(Src1 * C0)),
        reference=lambda in0, in1, s0, s1, imm2: np.minimum(
            in0.astype(np.float32) - 1.0,
            np.maximum(in1.astype(np.float32) * s0, 0.0),
        ),
    )
    row = max(dve_ops._SUB_OPCODE_FOR_NAME.values()) + 1
    assert row < 0x20
    shas = {}
    for ver in ("v3", "v4"):
        tmp = DveOpSpec(name=name, opcode=row, uops=lower(spec, ver=ver), rd1_en=True)
        shas[ver] = tmp.sha(ver)
    op = DveOp(name, spec, subdim=False, uops_sha=shas)
    OPS.append(op)
    dve_ops._SUB_OPCODE_FOR_NAME[name] = row
    dve_ops.CUSTOM_DVE_SPECS[name] = spec
    return op


ELU_TAIL = _register_elu_tail()

B, N, D = 8, 512, 1024
P = 128  # SBUF partitions
NB = N // P  # 4 row blocks
DB = D // P  # 8 contraction blocks
HW = D + N  # 1536: packed W|hT row length

F32 = mybir.dt.float32
U8 = mybir.dt.uint8
F16 = mybir.dt.float16
AF = mybir.ActivationFunctionType
ALU = mybir.AluOpType


def build_nc():
    nc = bacc.Bacc("TRN2", target_bir_lowering=False, debug=False, num_devices=B)

    # hw = [h^T | W] packed host-side: hw[d_row, 0:512] = h^T, [512:1536] = W
    hw = nc.dram_tensor("hw", [D, HW], F16, kind="ExternalInput").ap()
    adjT = nc.dram_tensor("adjT", [N, N], U8, kind="ExternalInput").ap()
    adjwT = nc.dram_tensor("adjwT", [N, N], F16, kind="ExternalInput").ap()
    out = nc.dram_tensor("out", [N, D], F16, kind="ExternalOutput").ap()
    out_r = out.rearrange("(n p) f -> p n f", p=P)     # [128, 4, 1024]
    hw_r = hw.rearrange("(n p) f -> p n f", p=P)       # [128, 8, 1536]
    adjT_r = adjT.rearrange("(n p) i -> p n i", p=P)   # [128, 4, 512]
    adjwT_r = adjwT.rearrange("(n p) i -> p n i", p=P)

    with tile.TileContext(nc) as tc:
        with (
            tc.tile_pool(name="singles", bufs=1) as singles,
            tc.tile_pool(name="work", bufs=10) as work,
            tc.tile_pool(name="outp", bufs=10) as outp,
            tc.tile_pool(name="psum", bufs=8, space="PSUM") as psum,
        ):
            # ---- resident SBUF tensors --------------------------------
            hw_sb = singles.tile([P, DB, HW], F16)   # [128, 8, 1536] = 3MB
            adjT_sb = singles.tile([P, NB, N], U8)
            adjw_sb = singles.tile([P, NB, N], F16)
            MT_sb = singles.tile([P, NB, N], F16)    # (adj * adj_weight)^T
            Wh_sb = singles.tile([P, NB, D], F16)    # [j-part, j-block, f]
            ones_w = singles.tile([P, 640], F16)     # warmup operands + ones
            t01 = singles.tile([P, N], F16)
            t23 = singles.tile([P, N], F16)
            S_sb = singles.tile([P, N], F16)         # sum over j-blocks of adjT
            r_sb = singles.tile([P, NB], F32)        # 1/deg
            exp_junk = singles.tile([P, 16], F32)

            # ---- input DMA: one ordered stream on the sync queue ------
            # Stream 1: [h | W-f0] per d (everything MM1-f0 needs), then
            # W-f1 in d-pairs, then adj.  MM2-f0 and its act tail overlap
            # MM1-f1 / MM2-f1.
            # d0 operands split across two queues (scalar queue is idle at
            # kernel start) so the DMA cold-start is paid in parallel
            nc.scalar.dma_start(hw_sb[:, 0, :512], hw_r[:, 0, :512])
            nc.sync.dma_start(hw_sb[:, 0, 512:1024], hw_r[:, 0, 512:1024])
            for d in range(1, DB):
                nc.sync.dma_start(hw_sb[:, d, :1024], hw_r[:, d, :1024])
            for dp in range(4):
                nc.sync.dma_start(
                    hw_sb[:, ts(dp, 2), 1024:], hw_r[:, ts(dp, 2), 1024:]
                )
            nc.sync.dma_start(adjT_sb, adjT_r)
            nc.sync.dma_start(adjw_sb, adjwT_r)

            # gpsimd memset is ready ~6.9us, well before the first dma lands
            nc.gpsimd.memset(ones_w, 1.0)
            # preload the ACT function table before the critical tail
            nc.scalar.activation(exp_junk, ones_w[:, :16], AF.Exp)

            # ---- PE MM1 f-split: all of f0 first, then f1 --------------
            ps1 = [psum.tile([P, 512], F32, name=f"ps1_{k}", tag="mm") for k in range(8)]

            # ---- PE warmup: sustained activity from ~7.5us so the HAM
            # clock gate flips to 2.4GHz right as the first data lands.
            # Warmups write ps1[0]; MM1's start=True then resets it.
            for _ in range(6):
                nc.tensor.matmul(
                    ps1[0], ones_w[:, :P], ones_w[:, P:640],
                    start=True, stop=True,
                )

            def mm1(d, f, j):
                nc.tensor.matmul(
                    ps1[f * NB + j],
                    hw_sb[:, d, j * P : (j + 1) * P],
                    hw_sb[:, d, N + f * 512 : N + (f + 1) * 512],
                    start=(d == 0),
                    stop=(d == DB - 1),
                )

            for d in range(DB):
                for j in range(NB):
                    mm1(d, 0, j)
            # evac f0 while MM1-f1 runs on the PE
            with tc.high_priority():
                nc.vector.tensor_copy(Wh_sb[:, 0, ts(0, 512)], ps1[0])
                nc.scalar.copy(Wh_sb[:, 1, ts(0, 512)], ps1[1])
                nc.vector.tensor_copy(Wh_sb[:, 2, ts(0, 512)], ps1[2])
                nc.scalar.copy(Wh_sb[:, 3, ts(0, 512)], ps1[3])

            for d in range(DB):
                for j in range(NB):
                    mm1(d, 1, j)
            with tc.high_priority():
                nc.vector.tensor_copy(Wh_sb[:, 0, ts(1, 512)], ps1[4])
                nc.scalar.copy(Wh_sb[:, 1, ts(1, 512)], ps1[5])
                nc.vector.tensor_copy(Wh_sb[:, 2, ts(1, 512)], ps1[6])
                nc.scalar.copy(Wh_sb[:, 3, ts(1, 512)], ps1[7])

            # ---- prep while MM1 runs: deg adds + MT (split V/G) -------
            nc.vector.tensor_add(t01, adjT_sb[:, 0], adjT_sb[:, 1])
            nc.vector.tensor_add(t23, adjT_sb[:, 2], adjT_sb[:, 3])
            nc.vector.tensor_add(S_sb, t01, t23)
            nc.gpsimd.tensor_mul(MT_sb[:, 0], adjT_sb[:, 0], adjw_sb[:, 0])
            nc.gpsimd.tensor_mul(MT_sb[:, 1], adjT_sb[:, 1], adjw_sb[:, 1])
            nc.vector.tensor_mul(MT_sb[:, 2], adjT_sb[:, 2], adjw_sb[:, 2])
            nc.vector.tensor_mul(MT_sb[:, 3], adjT_sb[:, 3], adjw_sb[:, 3])

            # deg via 4 one-column PE matmuls against ones (between MM1-f1
            # and MM2-f0 in the PE stream), then 1/deg on DVE.  deg_ps
            # takes ps1[0]'s bank, which the f0 evac freed long ago.
            deg_ps = psum.tile([P, NB], F32, tag="mm")
            for i in range(NB):
                nc.tensor.matmul(
                    deg_ps[:, i : i + 1],
                    S_sb[:, ts(i, P)],
                    ones_w[:, :1],
                    start=True,
                    stop=True,
                )
            with tc.high_priority():
                nc.vector.reciprocal(r_sb, deg_ps)

            # ---- PE MM2 + fused scale + ELU, f0 wave then f1 wave -----
            # x = r[i] * psum;  elu(x) = min(exp(x) - 1, relu(x))
            for f in range(2):
                for i in range(NB):
                    ps2 = psum.tile([P, 512], F32, name=f"ps2_{f}_{i}", tag="mm")
                    for j in range(NB):
                        nc.tensor.matmul(
                            ps2,
                            MT_sb[:, j, ts(i, P)],
                            Wh_sb[:, j, ts(f, 512)],
                            start=(j == 0),
                            stop=(j == NB - 1),
                        )
                    r_i = r_sb[:, i : i + 1]
                    last = i == NB - 1 and f == 1
                    # split the very last tile so the post-PE serial chain
                    # (exp -> elu -> dma) runs on half-width pieces
                    chunks = ((0, 256), (256, 512)) if last else ((0, 512),)
                    o_t = outp.tile([P, 512], F16)
                    for ci, (lo, hi) in enumerate(chunks):
                        w = hi - lo
                        exp_t = work.tile([P, 512], F16, tag="exp")
                        nc.scalar.activation(
                            exp_t[:, :w], ps2[:, lo:hi], AF.Exp, scale=r_i
                        )
                        nc.vector._custom_dve(
                            ELU_TAIL,
                            out=o_t[:, lo:hi],
                            in0=exp_t[:, :w],
                            in1=ps2[:, lo:hi],
                            s0=r_i,
                        )
                        q = nc.sync if (f + ci) % 2 == 0 else nc.gpsimd
                        q.dma_start(out_r[:, i, f * 512 + lo : f * 512 + hi], o_t[:, lo:hi])

    nc.compile()
    return nc


_NC = None


def _get_nc():
    global _NC
    if _NC is None:
        _NC = build_nc()
    return _NC


def _in_maps(h, adj, adj_weight, W):
    h = np.ascontiguousarray(np.asarray(h, dtype=np.float32))
    adj = np.asarray(adj)
    adj_weight = np.ascontiguousarray(np.asarray(adj_weight, dtype=np.float32))
    Wf = np.asarray(W, dtype=np.float32).reshape(D, D).astype(np.float16)
    hT = h.transpose(0, 2, 1).astype(np.float16)  # [B, 1024, 512]
    hw = np.concatenate([hT, np.broadcast_to(Wf, (B, D, D))], axis=2)
    hw = np.ascontiguousarray(hw)  # [B, 1024, 1536]
    adjT = np.ascontiguousarray(adj.transpose(0, 2, 1).astype(np.uint8))
    adjwT = np.ascontiguousarray(adj_weight.transpose(0, 2, 1).astype(np.float16))
    return [
        {"hw": hw[b], "adjT": adjT[b], "adjwT": adjwT[b]} for b in range(B)
    ]


def _run(h, adj, adj_weight, W, a=None, trace=False, **trace_kw):
    nc = _get_nc()
    res = run_bass_kernel_spmd(
        nc, _in_maps(h, adj, adj_weight, W), core_ids=list(range(B)),
        trace=trace, **trace_kw,
    )
    out = np.stack([res.results[c]["out"] for c in range(B)], axis=0)
    return out.astype(np.float32), res


def kernel(h, adj, adj_weight, W, a=None, **_ignored):
    # The NTFF trace path needs an axon hook module this container lacks;
    # make sure an ambient BASS_TRACE can't divert the graded run into it.
    os.environ["BASS_NEVER_TRACE"] = "1"
    out, _ = _run(h, adj, adj_weight, W)
    return out


# revision 77
# speedup vs baseline: 1.0065x; 1.0065x over previous
"""GAT kernel for Trainium2, SPMD over 8 NeuronCores.

Math: the reference GAT variant computes attention logits e[b,h,i,j] that do
NOT depend on j (the "untransposed Wh2" formulation), so softmax over a row
whose support (adj!=0) carries a constant value collapses to 1/deg(i) on the
support and 0 elsewhere (NEG_INF -> exp underflow -> exactly 0 in fp32).
Hence, per batch element b:

    out[b] = elu( diag(1/deg_b) @ (adj_b * adj_weight_b) @ (h_b @ W) )

with deg_b[i] = sum_j adj_b[i,j].  The result is head-independent and `a` is
unused.  Sharding: data-parallel over batch (B == n_cores == 8).

Schedule (v5):
 - W_d and h_d^T are concatenated host-side into one [1024, 1536] tensor so
   each contraction block d arrives as ONE dma (8 descriptor gens, d-order
   arrival); adj/adj_weight stream strictly after.
 - Tiny warmup matmuls on a gpsimd-memset tile trip the HAM clock gate
   before real data lands.
 - MM1 64 + MM2 64/2 matmuls at 512 cols; evac via DVE+ACT in j-order.
 - deg = DVE partial adds over adjT + 4 one-column PE matmuls vs ones.
 - ELU tail: one ACT exp + one custom DVE op per tile:
       out = min(exp_t - 1, relu(psum * r))
   (|r*x| < 0.5 on this data so exp never overflows f16).
 - Output f16 (upcast on host), store DMAs on the gpsimd queue.
"""

import os

import numpy as np

import concourse.bass as bass
import concourse.tile as tile
from concourse import bacc, mybir
from concourse.bass import ts
from concourse.bass_utils import run_bass_kernel_spmd

# ---- custom DVE op: ELU tail ---------------------------------------------
import concourse.dve_ops as dve_ops
from concourse.dve_ops import DveOp, OPS
from concourse.dve_spec import Spec, Src0, Src1, C0, One, relu, minn, lower
from concourse.dve_uop import DveOpSpec


def _register_elu_tail():
    name = "ELU_TAIL_ANT"
    for op in OPS:
        if op.name == name:
            return op
    spec = Spec(
        body=minn(Src0 - One, relu(Src1 * C0)),
        reference=lambda in0, in1, s0, s1, imm2: np.minimum(
            in0.astype(np.float32) - 1.0,
            np.maximum(in1.astype(np.float32) * s0, 0.0),
        ),
    )
    row = max(dve_ops._SUB_OPCODE_FOR_NAME.values()) + 1
    assert row < 0x20
    shas = {}
    for ver in ("v3", "v4"):
        tmp = DveOpSpec(name=name, opcode=row, uops=lower(spec, ver=ver), rd1_en=True)
        shas[ver] = tmp.sha(ver)
    op = DveOp(name, spec, subdim=False, uops_sha=shas)
    OPS.append(op)
    dve_ops._SUB_OPCODE_FOR_NAME[name] = row
    dve_ops.CUSTOM_DVE_SPECS[name] = spec
    return op


ELU_TAIL = _register_elu_tail()

B, N, D = 8, 512, 1024
P = 128  # SBUF partitions
NB = N // P  # 4 row blocks
DB = D // P  # 8 contraction blocks
HW = D + N  # 1536: packed W|hT row length

F32 = mybir.dt.float32
U8 = mybir.dt.uint8
F16 = mybir.dt.float16
AF = mybir.ActivationFunctionType
ALU = mybir.AluOpType


def build_nc():
    nc = bacc.Bacc("TRN2", target_bir_lowering=False, debug=False, num_devices=B)

    # hw = [h^T | W] packed host-side: hw[d_row, 0:512] = h^T, [512:1536] = W
    hw = nc.dram_tensor("hw", [D, HW], F16, kind="ExternalInput").ap()
    adjT = nc.dram_tensor("adjT", [N, N], U8, kind="ExternalInput").ap()
    adjwT = nc.dram_tensor("adjwT", [N, N], F16, kind="ExternalInput").ap()
    out = nc.dram_tensor("out", [N, D], F16, kind="ExternalOutput").ap()
    out_r = out.rearrange("(n p) f -> p n f", p=P)     # [128, 4, 1024]
    hw_r = hw.rearrange("(n p) f -> p n f", p=P)       # [128, 8, 1536]
    adjT_r = adjT.rearrange("(n p) i -> p n i", p=P)   # [128, 4, 512]
    adjwT_r = adjwT.rearrange("(n p) i -> p n i", p=P)

    with tile.TileContext(nc) as tc:
        with (
            tc.tile_pool(name="singles", bufs=1) as singles,
            tc.tile_pool(name="work", bufs=10) as work,
            tc.tile_pool(name="outp", bufs=10) as outp,
            tc.tile_pool(name="psum", bufs=8, space="PSUM") as psum,
        ):
            # ---- resident SBUF tensors --------------------------------
            hw_sb = singles.tile([P, DB, HW], F16)   # [128, 8, 1536] = 3MB
            adjT_sb = singles.tile([P, NB, N], U8)
            adjw_sb = singles.tile([P, NB, N], F16)
            MT_sb = singles.tile([P, NB, N], F16)    # (adj * adj_weight)^T
            Wh_sb = singles.tile([P, NB, D], F16)    # [j-part, j-block, f]
            ones_w = singles.tile([P, 640], F16)     # warmup operands + ones
            t01 = singles.tile([P, N], F16)
            t23 = singles.tile([P, N], F16)
            S_sb = singles.tile([P, N], F16)         # sum over j-blocks of adjT
            r_sb = singles.tile([P, NB], F32)        # 1/deg
            exp_junk = singles.tile([P, 16], F32)

            # ---- input DMA: one ordered stream on the sync queue ------
            # Stream 1: [h | W-f0] per d (everything MM1-f0 needs), then
            # W-f1 in d-pairs, then adj.  MM2-f0 and its act tail overlap
            # MM1-f1 / MM2-f1.
            # d0 operands split across two queues (scalar queue is idle at
            # kernel start) so the DMA cold-start is paid in parallel
            nc.scalar.dma_start(hw_sb[:, 0, :512], hw_r[:, 0, :512])
            nc.sync.dma_start(hw_sb[:, 0, 512:1024], hw_r[:, 0, 512:1024])
            for d in range(1, DB):
                nc.sync.dma_start(hw_sb[:, d, :1024], hw_r[:, d, :1024])
            for dp in range(4):
                nc.sync.dma_start(
                    hw_sb[:, ts(dp, 2), 1024:], hw_r[:, ts(dp, 2), 1024:]
                )
            nc.sync.dma_start(adjT_sb, adjT_r)
            nc.sync.dma_start(adjw_sb, adjwT_r)

            # gpsimd memset is ready ~6.9us, well before the first dma lands
            nc.gpsimd.memset(ones_w, 1.0)
            # preload the ACT function table before the critical tail
            nc.scalar.activation(exp_junk, ones_w[:, :16], AF.Exp)

            # ---- PE MM1 f-split: all of f0 first, then f1 --------------
            ps1 = [psum.tile([P, 512], F32, name=f"ps1_{k}", tag="mm") for k in range(8)]

            # ---- PE warmup: sustained activity from ~7.5us so the HAM
            # clock gate flips to 2.4GHz right as the first data lands.
            # Warmups write ps1[0]; MM1's start=True then resets it.
            for _ in range(6):
                nc.tensor.matmul(
                    ps1[0], ones_w[:, :P], ones_w[:, P:640],
                    start=True, stop=True,
                )

            def mm1(d, f, j):
                nc.tensor.matmul(
                    ps1[f * NB + j],
                    hw_sb[:, d, j * P : (j + 1) * P],
                    hw_sb[:, d, N + f * 512 : N + (f + 1) * 512],
                    start=(d == 0),
                    stop=(d == DB - 1),
                )

            for d in range(DB):
                for j in range(NB):
                    mm1(d, 0, j)
            # evac f0 while MM1-f1 runs on the PE
            with tc.high_priority():
                nc.vector.tensor_copy(Wh_sb[:, 0, ts(0, 512)], ps1[0])
                nc.scalar.copy(Wh_sb[:, 1, ts(0, 512)], ps1[1])
                nc.vector.tensor_copy(Wh_sb[:, 2, ts(0, 512)], ps1[2])
                nc.scalar.copy(Wh_sb[:, 3, ts(0, 512)], ps1[3])

            for d in range(DB):
                for j in range(NB):
                    mm1(d, 1, j)
            with tc.high_priority():
                nc.vector.tensor_copy(Wh_sb[:, 0, ts(1, 512)], ps1[4])
                nc.scalar.copy(Wh_sb[:, 1, ts(1, 512)], ps1[5])
                nc.vector.tensor_copy(Wh_sb[:, 2, ts(1, 512)], ps1[6])
                nc.scalar.copy(Wh_sb[:, 3, ts(1, 512)], ps1[7])

            # ---- prep while MM1 runs: deg adds + MT (split V/G) -------
            nc.vector.tensor_add(t01, adjT_sb[:, 0], adjT_sb[:, 1])
            nc.vector.tensor_add(t23, adjT_sb[:, 2], adjT_sb[:, 3])
            nc.vector.tensor_add(S_sb, t01, t23)
            nc.gpsimd.tensor_mul(MT_sb[:, 0], adjT_sb[:, 0], adjw_sb[:, 0])
            nc.gpsimd.tensor_mul(MT_sb[:, 1], adjT_sb[:, 1], adjw_sb[:, 1])
            # DVE MT products in halves: gives the scheduler a seam to slot
            # the high-priority reciprocal in as soon as deg lands
            for j in (2, 3):
                for lo in (0, 256):
                    nc.vector.tensor_mul(
                        MT_sb[:, j, lo : lo + 256],
                        adjT_sb[:, j, lo : lo + 256],
                        adjw_sb[:, j, lo : lo + 256],
                    )

            # deg via 4 one-column PE matmuls against ones (between MM1-f1
            # and MM2-f0 in the PE stream), then 1/deg on DVE.  deg_ps
            # takes ps1[0]'s bank, which the f0 evac freed long ago.
            deg_ps = psum.tile([P, NB], F32, tag="mm")
            for i in range(NB):
                nc.tensor.matmul(
                    deg_ps[:, i : i + 1],
                    S_sb[:, ts(i, P)],
                    ones_w[:, :1],
                    start=True,
                    stop=True,
                )
            with tc.high_priority():
                nc.vector.reciprocal(r_sb, deg_ps)

            # ---- PE MM2 + fused scale + ELU, f0 wave then f1 wave -----
            # x = r[i] * psum;  elu(x) = min(exp(x) - 1, relu(x))
            for f in range(2):
                for i in range(NB):
                    ps2 = psum.tile([P, 512], F32, name=f"ps2_{f}_{i}", tag="mm")
                    for j in range(NB):
                        nc.tensor.matmul(
                            ps2,
                            MT_sb[:, j, ts(i, P)],
                            Wh_sb[:, j, ts(f, 512)],
                            start=(j == 0),
                            stop=(j == NB - 1),
                        )
                    r_i = r_sb[:, i : i + 1]
                    last = i == NB - 1 and f == 1
                    # split the very last tile so the post-PE serial chain
                    # (exp -> elu -> dma) runs on half-width pieces
                    chunks = ((0, 256), (256, 512)) if last else ((0, 512),)
                    o_t = outp.tile([P, 512], F16)
                    for ci, (lo, hi) in enumerate(chunks):
                        w = hi - lo
                        exp_t = work.tile([P, 512], F16, tag="exp")
                        nc.scalar.activation(
                            exp_t[:, :w], ps2[:, lo:hi], AF.Exp, scale=r_i
                        )
                        nc.vector._custom_dve(
                            ELU_TAIL,
                            out=o_t[:, lo:hi],
                            in0=exp_t[:, :w],
                            in1=ps2[:, lo:hi],
                            s0=r_i,
                        )
                        q = nc.sync if (f + ci) % 2 == 0 else nc.gpsimd
                        q.dma_start(out_r[:, i, f * 512 + lo : f * 512 + hi], o_t[:, lo:hi])

    nc.compile()
    return nc


_NC = None


def _get_nc():
    global _NC
    if _NC is None:
        _NC = build_nc()
    return _NC


def _in_maps(h, adj, adj_weight, W):
    h = np.ascontiguousarray(np.asarray(h, dtype=np.float32))
    adj = np.asarray(adj)
    adj_weight = np.ascontiguousarray(np.asarray(adj_weight, dtype=np.float32))
    Wf = np.asarray(W, dtype=np.float32).reshape(D, D).astype(np.float16)
    hT = h.transpose(0, 2, 1).astype(np.float16)  # [B, 1024, 512]
    hw = np.concatenate([hT, np.broadcast_to(Wf, (B, D, D))], axis=2)
    hw = np.ascontiguousarray(hw)  # [B, 1024, 1536]
    adjT = np.ascontiguousarray(adj.transpose(0, 2, 1).astype(np.uint8))
    adjwT = np.ascontiguousarray(adj_weight.transpose(0, 2, 1).astype(np.float16))
    return [
        {"hw": hw[b], "adjT": adjT[b], "adjwT": adjwT[b]} for b in range(B)
    ]


def _run(h, adj, adj_weight, W, a=None, trace=False, **trace_kw):
    nc = _get_nc()
    res = run_bass_kernel_spmd(
        nc, _in_maps(h, adj, adj_weight, W), core_ids=list(range(B)),
        trace=trace, **trace_kw,
    )
    out = np.stack([res.results[c]["out"] for c in range(B)], axis=0)
    return out.astype(np.float32), res


def kernel(h, adj, adj_weight, W, a=None, **_ignored):
    # The NTFF trace path needs an axon hook module this container lacks;
    # make sure an ambient BASS_TRACE can't divert the graded run into it.
    os.environ["BASS_NEVER_TRACE"] = "1"
    out, _ = _run(h, adj, adj_weight, W)
    return out
